# revision 10
# baseline (speedup 1.0000x reference)
"""HBV hydrological model (nn_HBVMulTDET_WaterLoss) as a Bass/Tile kernel on
8 Trainium2 NeuronCores.

Strategy: pure data parallelism over the 4000 grid cells (500 cells/core).
Per-core layout: partition p in [0,125) holds 4 cells x 4 components = 16
state lanes in the free dim. All state-free derived quantities are
precomputed on the host and DMAd in ONE consolidated stream per time
chunk, so the device program is a pure steady-state recurrence stream:
the T=365 step loop fully unrolled and balanced across the DVE, Pool
(GPSIMD) and Scalar (ACT) engines by measured per-op cadence
(DVE ~90ns, Pool ~190ns, ACT ~215ns per op).

Key structural points vs a naive per-step translation:
  - snow melt/refreeze collapsed into one signed flux
        X = max(min(E, SP+SNOW), -MW),  E = melt_cap - refreeze_cap
    with meltwater carried negated (NMW); tosoil computed as
    NMWn - NMW2 (bit-identical to relu(NW - NMW2), one op less)
  - [SPn|NMW2] produced by ONE 32-lane tensor op (X broadcast over the
    pair); NMWn written directly into the next step's [SPa|NMW] tile
  - soil pow() via exp/ln with host-folded log constants:
    x1 = exp(BETA*ln(SM) - BLF), x2 = exp(BETAET*ln(SM1) + LNPB)
  - ET/SM update collapsed via SM3 = max(SMc - x2, max(SMc - PET, NZ))
  - response: rech+exc == SMa-SMc, (1-K) folding with negated states,
    [NSLZn|NSUZn] produced by ONE 32-lane mult against the adjacent
    [K2Cn|K1Cn] pair of the input stream, and Q0+Q1+Q2 accumulated in
    one strided-view tensor_reduce over the comb tile
All activations are forced into the single natural_log_exp_and_others
table set so the scalar engine never reloads its activation tables.
Gamma unit-hydrograph weights are computed on host; the routing
convolution runs on device.
"""
import math
import numpy as np

T_FULL = 365
NGRID = 4000
NCORES = 8
NSH = NGRID // NCORES      # 500 cells per core
PPART = 125                # partitions used
CL = 4                     # cells per partition
M = 4                      # nmul components
LENF = 15
NZ = 1e-5
TC = 32                    # time-chunk length
NST = 16                   # number of packed per-step streams

# stream order inside the packed dd tensor; K2Cn/K1Cn are adjacent and
# last so [NSLZn|NSUZn] = [K2Cn|K1Cn] * [SLZ2|SUZ3] is one 32-lane op
DD = ["SNOW", "E", "RAIN", "CWHn", "BETA", "BLF", "FC", "FCinv", "BETAET",
      "LNPB", "C", "PERC", "NUZL", "K0", "K2Cn", "K1Cn"]
DJ = {n: j for j, n in enumerate(DD)}

_TABLES_PATCHED = False


def _patch_act_tables():
    """Strip the functions of natural_log_exp_and_others from every other
    activation table set before the act-table-load CFG pass runs, so all
    activations resolve to that single set and the scalar engine loads its
    tables exactly once."""
    global _TABLES_PATCHED
    if _TABLES_PATCHED:
        return
    import concourse.bacc as bacc
    from concourse import hw_specs

    _orig = hw_specs.get_activation_tables
    target = "natural_log_exp_and_others"

    def _combined_only(arch):
        tables = _orig(arch)
        if target in tables:
            keep = tables[target]
            for name in list(tables):
                if name != target:
                    tables[name] = tables[name] - keep
        return tables

    bacc.get_activation_tables = _combined_only
    _TABLES_PATCHED = True


def build_program(T=T_FULL, tc_len=TC):
    _patch_act_tables()
    import concourse.bass as bass
    import concourse.bacc as bacc
    import concourse.mybir as mybir
    import concourse.tile as tile

    F32 = mybir.dt.float32
    op = mybir.AluOpType
    AF = mybir.ActivationFunctionType

    nc = bacc.Bacc("TRN2")
    dd = nc.declare_dram_parameter("dd", [PPART, T, NST, CL * M], F32,
                                   isOutput=False)
    pet = nc.declare_dram_parameter("pet", [PPART, T, CL], F32, isOutput=False)
    uh = nc.declare_dram_parameter("uh", [PPART, LENF * CL], F32, isOutput=False)
    qr = nc.declare_dram_parameter("qr", [PPART, T, CL], F32, isOutput=True)

    chunks = [(t0, min(tc_len, T - t0)) for t0 in range(0, T, tc_len)]

    with tile.TileContext(nc) as tctx:
        with (
            tctx.tile_pool(name="blk", bufs=2) as blk_pool,
            tctx.tile_pool(name="st", bufs=6) as st_pool,
            tctx.tile_pool(name="per", bufs=1) as per_pool,
        ):
            V = nc.vector
            G = nc.gpsimd
            A = nc.scalar
            S = nc.sync

            def tt(eng, out, a, b, o):
                eng.tensor_tensor(out, a, b, o)

            Qfull = per_pool.tile([PPART, (LENF - 1 + T) * CL], F32)
            uh_t = per_pool.tile([PPART, LENF * CL], F32)
            S.dma_start(uh_t[:], uh[:])
            G.memset(Qfull[:, : (LENF - 1) * CL], 0.0)

            # ---- state bootstrap ----
            SM = st_pool.tile([PPART, 16], F32, tag="SM")
            G.memset(SM[:], 0.001)
            # TM holds [SPn | NMW2]; at t=0 slot0 = SP0 (0.001)
            TM_prev = st_pool.tile([PPART, 32], F32, tag="TM")
            G.memset(TM_prev[:, 0:16], 0.001)
            # TSP holds [SPa | NMW]; slot1 is written by the previous step
            TSP_cur = st_pool.tile([PPART, 32], F32, tag="TSP")
            G.memset(TSP_cur[:, 16:32], -0.001)   # NMW init
            # comb: 8 slots of 16 lanes; lane = g*32 + x*16 (+4c+m):
            #   g0x0 SUZ2 | g1x0 SLZ2, g1x1 SUZ3 | g2x0 NSLZn | g3x0 NSUZn
            # so the Q reduce reads x=0 of a [p, x, c, g, m] view and the
            # [NSLZn|NSUZn] pair sits at uniform stride 32 for the merged
            # response mult.
            pc = st_pool.tile([PPART, 128], F32, tag="comb")
            G.memset(pc[:, 64:80], -0.001)    # NSLZ init
            G.memset(pc[:, 96:112], -0.001)   # NSUZ init

            def nt(tag, w=16):
                return st_pool.tile([PPART, w], F32, tag=tag, name=tag)

            def emit_dma(ci):
                t0, tcn = chunks[ci]
                dt_ = blk_pool.tile([PPART, tc_len * NST * 16], F32,
                                    tag="dd", name=f"dd_{t0}")
                S.dma_start(
                    dt_[:, : tcn * NST * 16].rearrange(
                        "p (t j f) -> p t j f", j=NST, f=16),
                    dd[:, t0 : t0 + tcn, :, :],
                )
                pt = blk_pool.tile([PPART, tc_len * CL], F32, tag="PET",
                                   name=f"PET_{t0}")
                S.dma_start(
                    pt[:, : tcn * CL].rearrange("p (t c) -> p t c", c=CL),
                    pet[:, t0 : t0 + tcn, :],
                )
                petb = (
                    pt[:, : tcn * CL]
                    .rearrange("p (t c) -> p t c", c=CL)
                    .unsqueeze(3)
                    .to_broadcast((PPART, tcn, CL, M))
                )
                return {"t0": t0, "tcn": tcn, "dt": dt_, "PETb": petb}

            cur = emit_dma(0)
            pendQ = None
            pendR = None

            def emit_pendR(p):
                """Deferred q-dependent response tail of the previous step,
                emitted inside the next step's snow window."""
                if p is None:
                    return
                cb = p["comb"]
                Q0 = nt("Q0")
                tt(G, Q0[:], p["K0"], p["q"][:], op.mult)
                SUZ3 = cb[:, 48:64]
                tt(G, SUZ3, cb[:, 0:16], Q0[:], op.subtract)
                # [NSLZn|NSUZn] = [K2Cn|K1Cn] * [SLZ2|SUZ3]  (one 32-lane op)
                tt(G,
                   cb[:, 64:128].rearrange("p (g f) -> p g f", g=2)[:, :, 0:16],
                   p["K1K2"],
                   cb[:, 32:64].rearrange("p (g f) -> p g f", g=2),
                   op.mult)

            def emit_pendQ(p):
                if p is None:
                    return
                # Q0+Q1+Q2 per cell = sum over {group, m} of
                # [SUZ2 | SLZ2 | NSLZn | NSUZn] — one strided-view reduce.
                V.tensor_reduce(
                    Qfull[:, (LENF - 1 + p["t"]) * CL : (LENF + p["t"]) * CL],
                    p["comb"][:].rearrange("p (g x c m) -> p x c g m",
                                           g=4, x=2, m=M)[:, 0],
                    axis=mybir.AxisListType.XY,
                    op=op.add,
                )

            for ci in range(len(chunks)):
                nxt = emit_dma(ci + 1) if ci + 1 < len(chunks) else None
                t0, tcn = cur["t0"], cur["tcn"]
                dt_ = cur["dt"]

                for ti in range(tcn):
                    t = t0 + ti

                    def cs(name):
                        j = DJ[name]
                        base = ti * NST * 16 + j * 16
                        return dt_[:, base : base + 16]

                    # ---- kick off the soil ACT chain for this step ----
                    lnSM = nt("lnSM")
                    A.activation(lnSM[:], SM[:], AF.Ln)

                    # ---- snow (Pool engine, fills the lnSM window) ----
                    tt(V, TSP_cur[:, 0:16], TM_prev[:, 0:16], cs("SNOW"),
                       op.add)               # SPa
                    mn = nt("mn")
                    tt(V, mn[:], cs("E"), TSP_cur[:, 0:16], op.min)
                    X = nt("X")
                    tt(V, X[:], mn[:], TSP_cur[:, 16:32], op.max)
                    TM = nt("TM", 32)         # [SPn | NMW2]
                    tt(V,
                       TM[:].rearrange("p (g f) -> p g f", g=2),
                       TSP_cur[:].rearrange("p (g f) -> p g f", g=2),
                       X[:].unsqueeze(1).to_broadcast((PPART, 2, 16)),
                       op.subtract)
                    NW = nt("NW")
                    tt(V, NW[:], cs("CWHn"), TM[:, 0:16], op.mult)
                    TSP_next = st_pool.tile([PPART, 32], F32, tag="TSP",
                                            name="TSP")
                    tt(V, TSP_next[:, 16:32], TM[:, 16:32], NW[:],
                       op.max)               # NMWn -> next step's NMW slot
                    tosp = nt("tosp")
                    tt(V, tosp[:], TSP_next[:, 16:32], TM[:, 16:32],
                       op.subtract)
                    wi = nt("wi")
                    tt(V, wi[:], cs("RAIN"), tosp[:], op.add)
                    TM_prev = TM
                    TSP_cur = TSP_next

                    # previous step's deferred response tail in this window
                    emit_pendR(pendR)
                    NSLZ = pc[:, 64:80]
                    NSUZ = pc[:, 96:112]
                    CnSLZ = nt("CnSLZ")
                    tt(G, CnSLZ[:], cs("C"), NSLZ, op.mult)

                    # ---- soil on-path ----
                    SMa = nt("SMa")
                    tt(V, SMa[:], SM[:], wi[:], op.add)
                    v = nt("v")
                    tt(V, v[:], lnSM[:], cs("BETA"), op.mult)
                    u = nt("u")
                    tt(V, u[:], v[:], cs("BLF"), op.subtract)
                    x1 = nt("x1")
                    A.activation(x1[:], u[:], AF.Exp)

                    # x1 window: previous step's Q output
                    emit_pendQ(pendQ)

                    rech = nt("rech")
                    V.scalar_tensor_tensor(rech[:], x1[:], 1.0, wi[:],
                                           op.min, op.mult)
                    SM1 = nt("SM1")
                    tt(V, SM1[:], SMa[:], rech[:], op.subtract)
                    ln2 = nt("ln2")
                    A.activation(ln2[:], SM1[:], AF.Ln)

                    # ln2 window: SMc, ET floor and the response head
                    SMc = nt("SMc")
                    tt(V, SMc[:], SM1[:], cs("FC"), op.min)
                    SMcP = nt("SMcP")
                    tt(G, SMcP[:].rearrange("p (c m) -> p c m", m=M),
                       SMc[:].rearrange("p (c m) -> p c m", m=M),
                       cur["PETb"][:, ti, :, :], op.subtract)
                    SMcP2 = nt("SMcP2")
                    V.tensor_scalar_max(SMcP2[:], SMcP[:], NZ)
                    SUZ1a = nt("SUZ1a")
                    tt(G, SUZ1a[:], SMa[:], NSUZ, op.subtract)
                    SUZ1 = nt("SUZ1")
                    tt(G, SUZ1[:], SUZ1a[:], SMc[:], op.subtract)
                    PERCa = nt("PERCa")
                    tt(V, PERCa[:], SUZ1[:], cs("PERC"), op.min)
                    comb = st_pool.tile([PPART, 128], F32, tag="comb",
                                        name="comb")
                    SUZ2 = comb[:, 0:16]
                    tt(G, SUZ2, SUZ1[:], PERCa[:], op.subtract)
                    t5 = nt("t5")
                    tt(G, t5[:], SUZ2, cs("NUZL"), op.add)
                    q = nt("q")
                    A.activation(q[:], t5[:], AF.Relu)

                    # ---- on-path: w2 = BETAET*ln2 + LNPB ----
                    v2 = nt("v2")
                    tt(V, v2[:], ln2[:], cs("BETAET"), op.mult)
                    w2 = nt("w2")
                    tt(V, w2[:], v2[:], cs("LNPB"), op.add)
                    x2 = nt("x2")
                    A.activation(x2[:], w2[:], AF.Exp)

                    # ---- on-path tail: SM3, capillary, SM ----
                    tq = nt("tq")
                    V.scalar_tensor_tensor(tq[:], x2[:], -1.0, SMc[:],
                                           op.mult, op.add)
                    SM3 = nt("SM3")
                    tt(V, SM3[:], tq[:], SMcP2[:], op.max)
                    g = nt("g")
                    tt(V, g[:], SM3[:], cs("FCinv"), op.mult)
                    rln = nt("rln")
                    V.tensor_scalar(rln[:], g[:], 1.0, 1.0, op.min,
                                    op.subtract)
                    cap = nt("cap")
                    tt(V, cap[:], CnSLZ[:], rln[:], op.mult)
                    SMn = nt("SM")
                    tt(V, SMn[:], SM3[:], cap[:], op.add)
                    SM = SMn

                    # ---- response tail ----
                    sl_n = nt("sl_n")
                    tt(G, sl_n[:], NSLZ, cap[:], op.add)
                    NSLZ1 = nt("NSLZ1")
                    V.tensor_scalar_min(NSLZ1[:], sl_n[:], -NZ)
                    SLZ2 = comb[:, 32:48]
                    tt(G, SLZ2, PERCa[:], NSLZ1[:], op.subtract)

                    base = ti * NST * 16 + DJ["K2Cn"] * 16
                    K1K2 = dt_[:, base : base + 32].rearrange(
                        "p (g f) -> p g f", g=2)
                    pendR = {"comb": comb, "q": q, "K0": cs("K0"),
                             "K1K2": K1K2}
                    pendQ = {"t": t, "comb": comb}
                    pc = comb

                if nxt is not None:
                    cur = nxt

            emit_pendR(pendR)
            emit_pendQ(pendQ)

            # ---- gamma-UH routing (DVE, bulk) ----
            Qr = per_pool.tile([PPART, T * CL], F32)
            prod = per_pool.tile([PPART, T * CL], F32)

            def qr4(ap_):
                return ap_.rearrange("p (t c) -> p t c", c=CL)

            for k in range(LENF):
                sh = Qfull[:, (LENF - 1 - k) * CL : (LENF - 1 - k + T) * CL]
                uhk = (
                    uh_t[:, k * CL : (k + 1) * CL]
                    .unsqueeze(1)
                    .to_broadcast((PPART, T, CL))
                )
                if k == 0:
                    tt(V, qr4(Qr[:]), uhk, qr4(sh), op.mult)
                else:
                    tt(V, qr4(prod[:]), uhk, qr4(sh), op.mult)
                    tt(V, qr4(Qr[:]), qr4(Qr[:]), qr4(prod[:]), op.add)

            S.dma_start(qr[:, :, :], Qr[:].rearrange("p (t c) -> p t c", c=CL))

    return nc


# ---------------- host-side packing ----------------

def _derived_full(x_hydro_model, params_raw):
    """All state-free per-step tensors, float32, shapes [T, N, M] (per-cell
    quantities broadcast over M)."""
    f32 = np.float32
    T, N, _ = x_hydro_model.shape
    raw = np.ascontiguousarray(params_raw[:, :, :14, :], dtype=f32)
    x = np.ascontiguousarray(x_hydro_model, dtype=f32)
    P = x[:, :, 0:1]
    Ta = x[:, :, 1:2]
    PET = x[:, :, 2:3]

    BETA = f32(5.0) * raw[:, :, 0] + f32(1.0)
    FC = f32(950.0) * raw[:, :, 1] + f32(50.0)
    K0 = f32(0.85) * raw[:, :, 2] + f32(0.05)
    K1Cn = f32(0.49) * raw[:, :, 3] - f32(0.99)
    K2Cn = f32(0.199) * raw[:, :, 4] - f32(0.999)
    LP = f32(0.8) * raw[:, :, 5] + f32(0.2)
    PERC = f32(10.0) * raw[:, :, 6]
    NUZL = f32(-100.0) * raw[:, :, 7]
    TTn = f32(-5.0) * raw[:, :, 8] + f32(2.5)
    CFMX = f32(9.5) * raw[:, :, 9] + f32(0.5)
    CWHn = f32(-0.2) * raw[:, :, 11]
    BETAET = f32(4.7) * raw[:, :, 12] + f32(0.3)
    C = raw[:, :, 13]

    Tdiff = (Ta + TTn).astype(f32)
    m1 = (CFMX * Tdiff).astype(f32)
    rn = np.maximum(-m1, 0).astype(f32)
    Rc0 = ((f32(0.1) * raw[:, :, 10]).astype(f32) * rn).astype(f32)
    Gc0 = np.maximum(m1, 0).astype(f32)
    E = (Gc0 - Rc0).astype(f32)
    mask = (Tdiff >= 0).astype(f32)
    RAIN = (mask * P).astype(f32)
    SNOW = (P - RAIN).astype(f32)
    lnFC = np.log(FC).astype(f32)
    FCinv = np.exp(-lnFC).astype(f32)
    BLF = (BETA * lnFC).astype(f32)
    LPFC = (LP * FC).astype(f32)
    lnLPFC = np.log(LPFC).astype(f32)
    BL2 = (BETAET * lnLPFC).astype(f32)
    lnPET = np.log(np.maximum(PET, f32(1e-30))).astype(f32)
    LNPB = (lnPET - BL2).astype(f32)

    return {
        "E": E, "SNOW": SNOW, "RAIN": RAIN, "CWHn": CWHn, "BETA": BETA,
        "BLF": BLF, "FC": FC, "FCinv": FCinv, "BETAET": BETAET, "LNPB": LNPB,
        "C": C, "PERC": PERC, "NUZL": NUZL, "K0": K0, "K1Cn": K1Cn,
        "K2Cn": K2Cn,
    }


def pack_inputs(x_hydro_model, params_raw, conv_params_hydro):
    T = x_hydro_model.shape[0]
    f32 = np.float32
    der = _derived_full(x_hydro_model, params_raw)
    # [T, N, M] -> per core [PPART, T, NST, CL*M]
    dd_full = np.stack([der[n] for n in DD], axis=0)  # [nd, T, N, M]
    nd = dd_full.shape[0]
    dd_c = (dd_full.reshape(nd, T, NCORES, PPART, CL * M)
            .transpose(2, 3, 1, 0, 4))           # [cores, P, T, nd, 16]

    PET = np.ascontiguousarray(x_hydro_model[:, :, 2], dtype=f32)  # [T, N]
    pet_c = PET.reshape(T, NCORES, PPART, CL).transpose(1, 2, 0, 3)

    conv = np.asarray(conv_params_hydro, dtype=np.float64)
    a = conv[:, 0] * 2.9
    b = conv[:, 1] * 6.5
    aa = np.maximum(a, 0) + 0.1
    theta = np.maximum(b, 0) + 0.5
    tgrid = np.arange(0.5, float(LENF), dtype=np.float64)[:, None]
    lg = np.array([math.lgamma(v) for v in aa])
    w = np.exp(-lg) / theta ** aa * tgrid ** (aa - 1.0) * np.exp(-tgrid / theta)
    w = w / w.sum(0)
    UH = (w * (1.0 / M)).astype(f32)  # [LENF, NGRID], mean-over-M folded in
    uh_c = UH.reshape(LENF, NCORES, PPART, CL).transpose(1, 2, 0, 3)

    in_maps = []
    for i in range(NCORES):
        in_maps.append({
            "dd": np.ascontiguousarray(dd_c[i]),
            "pet": np.ascontiguousarray(pet_c[i]),
            "uh": np.ascontiguousarray(uh_c[i]).reshape(PPART, LENF * CL),
        })
    return in_maps


def unpack_outputs(results, T):
    out = np.empty((T, NGRID), np.float32)
    for i in range(NCORES):
        q = results[i]["qr"].reshape(PPART, T, CL)
        out[:, i * NSH : (i + 1) * NSH] = q.transpose(1, 0, 2).reshape(T, NSH)
    return out


_PROG_CACHE = {}


def kernel(x_hydro_model, params_raw, conv_params_hydro):
    from concourse.bass_utils import run_bass_kernel_spmd

    T = x_hydro_model.shape[0]
    key = T
    if key not in _PROG_CACHE:
        _PROG_CACHE[key] = build_program(T=T)
    nc = _PROG_CACHE[key]
    if not nc.is_finalized():
        nc.finalize()
    in_maps = pack_inputs(x_hydro_model, params_raw, conv_params_hydro)
    res = run_bass_kernel_spmd(nc, in_maps, list(range(NCORES)))
    return unpack_outputs(res.results, T)
